# revision 23
# baseline (speedup 1.0000x reference)
"""Multi-head attention kernel for Trainium2, SPMD over 8 NeuronCores.

Problem: B=4, S=4096, E=256, H=4 heads (dh=64), f32.
  q = target @ Wq ; k = source @ Wk ; v = source @ Wv   (per-head slices)
  out = softmax(q k^T / sqrt(dh)) v  -> concat heads -> @ Wo

Sharding: core c handles batch b = c//2 and heads (2*(c%2), 2*(c%2)+1).
Each core computes, for its two heads, the transposed unnormalized
attention output u^T = V^T @ exp(K Q^T / 8) of shape [64, 4096] plus the
softmax denominators (via a ones-column appended to V inside the PV
matmul). Host applies the cheap parts: QKV projections (tiny GEMMs),
the final normalization, and the output projection + cross-head sum.

Device data layout (per core):
  qt: [128, 4096] bf16  rows 0-63 head-A Q^T, rows 64-127 head-B Q^T
  kt: [128, 4096] bf16  same for K^T
  v:  [128, 2, 32, 65] bf16  v[p,h,j,0:64] = V[j*128+p, :]; [...,64] = 1
  out:[2, 65, 4096] f32  out[h,0:64,q] = u_h^T, out[h,64,q] = sum_k exp

v3 over the 271us v2: the v2 trace showed the ACT exp and DVE
(Schraudolph) exp were fully SERIALIZED (DVE started exactly when ACT
ended, every step) because both read halves of one shared PSUM tile --
the Tile tracker treated the two reads as conflicting. Fix: the two
heads' QK matmuls now write two SEPARATE PSUM tiles (one pool per
head). ACT only ever touches head-A scores, DVE only head-B, so the
engines run concurrently and each head's QK matmul waits only on its
own reader.
"""

import numpy as np

S = 4096
E = 256
H = 4
DH = 64
NJ = S // 128   # 32 key chunks of 128
QB = 512        # q block width (per head, per step)
NQB = S // QB   # 8
NCORES = 8

import os
EVAC = os.environ.get("K_EVAC", "split")   # "dve" | "split" (ACT+DVE)
EXBUFS = int(os.environ.get("K_EXBUFS", "8"))
QKBUFS = int(os.environ.get("K_QKBUFS", "2"))
DEPTH = int(os.environ.get("K_DEPTH", "3"))  # exp->PV software-pipeline depth
WARM = int(os.environ.get("K_WARM", "34"))   # garbage QK-shaped pairs to warm the PE clock
WARMN = int(os.environ.get("K_WARMN", "128"))  # moving width of warm pairs
EVDT = os.environ.get("K_EVDT", "bf16")      # evac/output dtype: bf16 | f32

# Schraudolph exp constants for bf16-bits-as-int16:
#   bits16(exp(s/8)) ~= s * (2^7 * log2(e) / 8) + 2^7 * (127 - 0.0573)
SCH_A = 128.0 * 1.4426950408889634 / 8.0   # 23.0831206...
SCH_B = 128.0 * 127.0 - 7.33               # 16248.67

_CACHE = {}


def _build_nc():
    import concourse.mybir as mybir
    import concourse.tile as tile
    from concourse import bacc

    f32 = mybir.dt.float32
    bf16 = mybir.dt.bfloat16
    i16 = mybir.dt.int16
    EXP = mybir.ActivationFunctionType.Exp
    MULT = mybir.AluOpType.mult
    ADD = mybir.AluOpType.add

    nc = bacc.Bacc("TRN2", target_bir_lowering=False, debug=False)

    evdt = bf16 if EVDT == "bf16" else f32
    qt_d = nc.dram_tensor("qt", [128, S], bf16, kind="ExternalInput").ap()
    kt_d = nc.dram_tensor("kt", [128, S], bf16, kind="ExternalInput").ap()
    v_d = nc.dram_tensor("v", [128, 2, NJ, 65], bf16, kind="ExternalInput").ap()
    out_d = nc.dram_tensor("out", [2, 65, S], evdt, kind="ExternalOutput").ap()

    # Everything runs in the PE's 64x128 row-tiled mode (contraction 64,
    # two concurrent tiles at partition offsets 0 and 64) — no 128-mode
    # matmuls anywhere, so the PE never drains for a tiling-mode switch
    # and the HAM clock ramps to 2.4 GHz.
    #   QK: head A on tile (0,0), head B on tile (64,0), same j chunk.
    #   PV: per step the two concurrent pairs are (A,part0)||(A,part1)
    #       then (B,part0)||(B,part1); each (head, part) accumulates into
    #       its own [65, QB] PSUM tile over the 32 j chunks.
    with tile.TileContext(nc) as tc:
        with (
            tc.tile_pool(name="const", bufs=1) as const,
            tc.tile_pool(name="exap", bufs=EXBUFS) as exap,
            tc.tile_pool(name="exbp", bufs=EXBUFS) as exbp,
            tc.tile_pool(name="evp", bufs=2) as evp,
            tc.tile_pool(name="qkap", bufs=QKBUFS, space="PSUM") as qkap,
            tc.tile_pool(name="qkbp", bufs=QKBUFS, space="PSUM") as qkbp,
            tc.tile_pool(name="pvp", bufs=1, space="PSUM") as pvp,
        ):
            # DMA order follows first-use order in the pipeline: the first
            # QK needs only qt block 0 and kt chunks 0-1; PV starts consuming
            # v chunks from step 2 (~13us), so v's first half must not queue
            # behind the qt/kt remainders (that cost ~4.5us of early ACT
            # stalls when v was emitted last).
            qt = const.tile([128, S], bf16)
            kt = const.tile([128, S], bf16)
            vsb = const.tile([128, 2, NJ, 65], bf16)

            # Warm-up, overlapped with the ~7us input-DMA wait (emitted
            # before the dma_starts so nothing gates it):
            #  - garbage matmuls from a never-written SBUF tile keep the PE
            #    busy so the HAM clock gate opens (1.2->2.4GHz) before the
            #    first real QK instead of during the first ~15 steps;
            #  - a tiny Exp activation pulls the ~1.3us ACT_TABLE_LOAD off
            #    the first real exp's critical path.
            if WARM:
                # Warm pairs must look like the real QK pairs (two
                # concurrent 64x128 row-tiles = full array) -- a lone
                # 64x65 matmul stream does NOT trip the HAM un-throttle
                # (measured: 24 such MMs ran 10us all-cold). They end
                # before the input DMA lands so they cost nothing.
                wsrc = const.tile([128, 128], bf16)
                wact = const.tile([1, 4], bf16)
                nc.vector.memset(wsrc[:, :], 0.0)
                nc.scalar.activation(wact[0:1, 0:4], wsrc[0:1, 0:4], EXP)
                wqa = qkap.tile([128, QB], f32, tag="qka", name="warm_a")
                wqb = qkbp.tile([128, QB], f32, tag="qkb", name="warm_b")
                for _ in range(WARM):
                    for qk, psl in ((wqa, slice(0, 64)), (wqb, slice(64, 128))):
                        nc.tensor.matmul(
                            qk[:, 0:WARMN], wsrc[psl, :], wsrc[psl, 0:WARMN],
                            start=True, stop=True,
                        )

            # qt block 0 first: it is the largest chunk the first QK waits
            # on, and the per-dma_start trigger costs ~700ns each on the
            # Sync queue, so issue order sets arrival order.
            nc.sync.dma_start(qt[:, 0:QB], qt_d[:, 0:QB])
            nc.sync.dma_start(kt[:, 0:128], kt_d[:, 0:128])
            nc.sync.dma_start(kt[:, 128:512], kt_d[:, 128:512])
            nc.sync.dma_start(vsb[:, :, 0:16, :], v_d[:, :, 0:16, :])
            nc.sync.dma_start(kt[:, 512:], kt_d[:, 512:])
            nc.sync.dma_start(vsb[:, :, 16:, :], v_d[:, :, 16:, :])
            nc.sync.dma_start(qt[:, QB:], qt_d[:, QB:])

            # Software-pipelined emission. PE stream per step is
            #   [PV(s-2) pair1] [QK(s) pair] [PV(s-2) pair2]
            # so that every LDWEIGHTS is two 215ns slots behind the matmul
            # whose weight buffer it reuses (each row group has 3 loads per
            # step and only fg+bg weight slots; an MM's full completion lags
            # its 215ns streaming slot by ~160ns, so a 1-slot distance —
            # QK emitted back-to-back after PV — stalled ~300ns per step).
            steps = [(qb, j) for qb in range(NQB) for j in range(NJ)]
            exs = {}
            pv = {}
            pend = []

            def emit_evac(item):
                h, p0, p1, qb2 = item
                qsl2 = slice(qb2 * QB, (qb2 + 1) * QB)
                ev = evp.tile([65, QB], evdt)
                if EVAC == "split":
                    nc.scalar.copy(ev[:], p0[:, :])
                else:
                    nc.vector.tensor_copy(ev[:], p0[:, :])
                nc.vector.tensor_add(ev[:], ev[:], p1[:, :])
                nc.sync.dma_start(out_d[h, :, qsl2], ev[:])

            def pv_mms(s, pairs):
                qb, j = steps[s]
                exa, exb = exs[s]
                for h, part in pairs:
                    psl = slice(part * 64, (part + 1) * 64)
                    tgt = pv[h, part]
                    first = j == 0
                    last = j == NJ - 1
                    mv = exa[psl, :] if h == 0 else exb[psl, :].bitcast(bf16)
                    nc.tensor.matmul(
                        tgt[:, :],
                        vsb[psl, h, j, :],
                        mv,
                        start=first,
                        stop=last,
                    )

            for s in range(len(steps) + DEPTH):
                if s >= DEPTH:
                    qb2, j2 = steps[s - DEPTH]
                    if j2 == 0:
                        for h in range(2):
                            for part in range(2):
                                pv[h, part] = pvp.tile(
                                    [65, QB], f32,
                                    tag=f"pv_{h}_{part}",
                                    name=f"pv_{h}_{part}",
                                )
                    # pair1: row groups 0 / 64
                    pv_mms(s - DEPTH, ((0, 0), (0, 1)))
                if s < len(steps):
                    qb, j = steps[s]
                    qsl = slice(qb * QB, (qb + 1) * QB)
                    jsl = slice(j * 128, (j + 1) * 128)
                    qka = qkap.tile([128, QB], f32, tag="qka")
                    qkb = qkbp.tile([128, QB], f32, tag="qkb")
                    # head B first: DVE (the slower exp engine) gets its
                    # input one MM-tail (~33ns) earlier.
                    for h, qk in ((1, qkb), (0, qka)):
                        psl = slice(h * 64, (h + 1) * 64)
                        nc.tensor.matmul(
                            qk[:, :],
                            kt[psl, jsl],
                            qt[psl, qsl],
                            start=True,
                            stop=True,
                        )
                    # Head A's exp -> bf16 tile via ACT (true Exp); head B's
                    # -> int16 tile via DVE (Schraudolph). Each engine reads
                    # its own PSUM tile so they run concurrently.
                    exa = exap.tile([128, QB], bf16)
                    exb = exbp.tile([128, QB], i16)
                    nc.vector.tensor_scalar(
                        out=exb[:],
                        in0=qkb[:, :],
                        scalar1=SCH_A,
                        scalar2=SCH_B,
                        op0=MULT,
                        op1=ADD,
                    )
                    nc.scalar.activation(
                        exa[:], qka[:, :], EXP, scale=0.125
                    )
                    exs[s] = (exa, exb)
                # Deferred evacs: one per step so the copy/add pair doesn't
                # burst both heads into the ACT/DVE queues at once (which
                # delayed the exp stream ~1.7us per block boundary). The
                # head-1 evac of block b lands here, which in emission order
                # is BEFORE pair2 of (b+1, j=0) -- i.e. before block b's pv
                # banks are rewritten -- so the read-before-write order the
                # tile tracker derives is correct.
                if pend:
                    emit_evac(pend.pop(0))
                if s >= DEPTH:
                    qb2, j2 = steps[s - DEPTH]
                    # pair2
                    pv_mms(s - DEPTH, ((1, 0), (1, 1)))
                    exs.pop(s - DEPTH)
                    if j2 == NJ - 1:
                        pend.append((0, pv[0, 0], pv[0, 1], qb2))
                        pend.append((1, pv[1, 0], pv[1, 1], qb2))
                        emit_evac(pend.pop(0))
            while pend:
                emit_evac(pend.pop(0))

    nc.compile()
    return nc


def _get_nc():
    if "nc" not in _CACHE:
        _CACHE["nc"] = _build_nc()
    return _CACHE["nc"]


def kernel(target, source, Wq, Wk, Wv, Wo):
    from concourse.bass_utils import run_bass_kernel_spmd

    target = np.asarray(target, dtype=np.float32)
    source = np.asarray(source, dtype=np.float32)
    Wq = np.asarray(Wq, dtype=np.float32)
    Wk = np.asarray(Wk, dtype=np.float32)
    Wv = np.asarray(Wv, dtype=np.float32)
    Wo = np.asarray(Wo, dtype=np.float32)
    B = target.shape[0]

    import ml_dtypes

    bf16 = ml_dtypes.bfloat16
    in_maps = []
    for c in range(NCORES):
        b = c // 2
        h0 = (c % 2) * 2
        cols = slice(h0 * DH, (h0 + 2) * DH)  # 128 cols = 2 heads
        q = target[b] @ Wq[:, cols]           # [S, 128]
        k = source[b] @ Wk[:, cols]           # [S, 128]
        v = source[b] @ Wv[:, cols]           # [S, 128]
        vv = np.ones((128, 2, NJ, 65), bf16)
        # v[p, h, j, 0:64] = V[j*128+p, h*64:(h+1)*64]
        vr = v.reshape(NJ, 128, 2, DH)        # [j, p, h, d]
        vv[:, :, :, :DH] = vr.transpose(1, 2, 0, 3).astype(bf16)
        in_maps.append(
            {
                "qt": np.ascontiguousarray(q.T.astype(bf16)),
                "kt": np.ascontiguousarray(k.T.astype(bf16)),
                "v": vv,
            }
        )

    nc = _get_nc()
    res = run_bass_kernel_spmd(nc, in_maps, core_ids=list(range(NCORES)))

    out = np.zeros((B, S, E), np.float32)
    for c in range(NCORES):
        b = c // 2
        h0 = (c % 2) * 2
        u = np.asarray(res.results[c]["out"], dtype=np.float32)  # [2, 65, S]
        for hh in range(2):
            att_t = u[hh, :DH, :] / u[hh, DH:DH + 1, :]   # [64, S]
            out[b] += att_t.T @ Wo[(h0 + hh) * DH:(h0 + hh + 1) * DH, :]
    return out


# revision 24
# speedup vs baseline: 1.0134x; 1.0134x over previous
"""Multi-head attention kernel for Trainium2, SPMD over 8 NeuronCores.

Problem: B=4, S=4096, E=256, H=4 heads (dh=64), f32.
  q = target @ Wq ; k = source @ Wk ; v = source @ Wv   (per-head slices)
  out = softmax(q k^T / sqrt(dh)) v  -> concat heads -> @ Wo

Sharding: core c handles batch b = c//2 and heads (2*(c%2), 2*(c%2)+1).
Each core computes, for its two heads, the transposed unnormalized
attention output u^T = V^T @ exp(K Q^T / 8) of shape [64, 4096] plus the
softmax denominators (via a ones-column appended to V inside the PV
matmul). Host applies the cheap parts: QKV projections (tiny GEMMs),
the final normalization, and the output projection + cross-head sum.

Device data layout (per core):
  qt: [128, 4096] bf16  rows 0-63 head-A Q^T, rows 64-127 head-B Q^T
  kt: [128, 4096] bf16  same for K^T
  v:  [128, 2, 32, 65] bf16  v[p,h,j,0:64] = V[j*128+p, :]; [...,64] = 1
  out:[2, 65, 4096] f32  out[h,0:64,q] = u_h^T, out[h,64,q] = sum_k exp

v3 over the 271us v2: the v2 trace showed the ACT exp and DVE
(Schraudolph) exp were fully SERIALIZED (DVE started exactly when ACT
ended, every step) because both read halves of one shared PSUM tile --
the Tile tracker treated the two reads as conflicting. Fix: the two
heads' QK matmuls now write two SEPARATE PSUM tiles (one pool per
head). ACT only ever touches head-A scores, DVE only head-B, so the
engines run concurrently and each head's QK matmul waits only on its
own reader.
"""

import numpy as np

S = 4096
E = 256
H = 4
DH = 64
NJ = S // 128   # 32 key chunks of 128
QB = 512        # q block width (per head, per step)
NQB = S // QB   # 8
NCORES = 8

import os
EVAC = os.environ.get("K_EVAC", "split")   # "dve" | "split" (ACT+DVE)
EXBUFS = int(os.environ.get("K_EXBUFS", "8"))
QKBUFS = int(os.environ.get("K_QKBUFS", "2"))
DEPTH = int(os.environ.get("K_DEPTH", "3"))  # exp->PV software-pipeline depth
WARM = int(os.environ.get("K_WARM", "34"))   # garbage QK-shaped pairs to warm the PE clock
WARMN = int(os.environ.get("K_WARMN", "128"))  # moving width of warm pairs
EVDT = os.environ.get("K_EVDT", "bf16")      # evac/output dtype: bf16 | f32

# Schraudolph exp constants for bf16-bits-as-int16:
#   bits16(exp(s/8)) ~= s * (2^7 * log2(e) / 8) + 2^7 * (127 - 0.0573)
SCH_A = 128.0 * 1.4426950408889634 / 8.0   # 23.0831206...
SCH_B = 128.0 * 127.0 - 7.33               # 16248.67

_CACHE = {}


def _build_nc():
    import concourse.mybir as mybir
    import concourse.tile as tile
    from concourse import bacc

    f32 = mybir.dt.float32
    bf16 = mybir.dt.bfloat16
    i16 = mybir.dt.int16
    EXP = mybir.ActivationFunctionType.Exp
    MULT = mybir.AluOpType.mult
    ADD = mybir.AluOpType.add

    nc = bacc.Bacc("TRN2", target_bir_lowering=False, debug=False)

    evdt = bf16 if EVDT == "bf16" else f32
    qt_d = nc.dram_tensor("qt", [128, S], bf16, kind="ExternalInput").ap()
    kt_d = nc.dram_tensor("kt", [128, S], bf16, kind="ExternalInput").ap()
    v_d = nc.dram_tensor("v", [128, 2, NJ, 65], bf16, kind="ExternalInput").ap()
    out_d = nc.dram_tensor("out", [2, 65, S], evdt, kind="ExternalOutput").ap()

    # Everything runs in the PE's 64x128 row-tiled mode (contraction 64,
    # two concurrent tiles at partition offsets 0 and 64) — no 128-mode
    # matmuls anywhere, so the PE never drains for a tiling-mode switch
    # and the HAM clock ramps to 2.4 GHz.
    #   QK: head A on tile (0,0), head B on tile (64,0), same j chunk.
    #   PV: per step the two concurrent pairs are (A,part0)||(A,part1)
    #       then (B,part0)||(B,part1); each (head, part) accumulates into
    #       its own [65, QB] PSUM tile over the 32 j chunks.
    with tile.TileContext(nc) as tc:
        with (
            tc.tile_pool(name="const", bufs=1) as const,
            tc.tile_pool(name="exap", bufs=EXBUFS) as exap,
            tc.tile_pool(name="exbp", bufs=EXBUFS) as exbp,
            tc.tile_pool(name="evp", bufs=2) as evp,
            tc.tile_pool(name="qkap", bufs=QKBUFS, space="PSUM") as qkap,
            tc.tile_pool(name="qkbp", bufs=QKBUFS, space="PSUM") as qkbp,
            tc.tile_pool(name="pvp", bufs=1, space="PSUM") as pvp,
        ):
            # DMA order follows first-use order in the pipeline: the first
            # QK needs only qt block 0 and kt chunks 0-1; PV starts consuming
            # v chunks from step 2 (~13us), so v's first half must not queue
            # behind the qt/kt remainders (that cost ~4.5us of early ACT
            # stalls when v was emitted last).
            qt = const.tile([128, S], bf16)
            kt = const.tile([128, S], bf16)
            vsb = const.tile([128, 2, NJ, 65], bf16)

            # Warm-up, overlapped with the ~7us input-DMA wait (emitted
            # before the dma_starts so nothing gates it):
            #  - garbage matmuls from a never-written SBUF tile keep the PE
            #    busy so the HAM clock gate opens (1.2->2.4GHz) before the
            #    first real QK instead of during the first ~15 steps;
            #  - a tiny Exp activation pulls the ~1.3us ACT_TABLE_LOAD off
            #    the first real exp's critical path.
            if WARM:
                # Warm pairs must look like the real QK pairs (two
                # concurrent 64x128 row-tiles = full array) -- a lone
                # 64x65 matmul stream does NOT trip the HAM un-throttle
                # (measured: 24 such MMs ran 10us all-cold). They end
                # before the input DMA lands so they cost nothing.
                wsrc = const.tile([128, 128], bf16)
                wact = const.tile([1, 4], bf16)
                nc.vector.memset(wsrc[:, :], 0.0)
                nc.scalar.activation(wact[0:1, 0:4], wsrc[0:1, 0:4], EXP)
                wqa = qkap.tile([128, QB], f32, tag="qka", name="warm_a")
                wqb = qkbp.tile([128, QB], f32, tag="qkb", name="warm_b")
                for _ in range(WARM):
                    for qk, psl in ((wqa, slice(0, 64)), (wqb, slice(64, 128))):
                        nc.tensor.matmul(
                            qk[:, 0:WARMN], wsrc[psl, :], wsrc[psl, 0:WARMN],
                            start=True, stop=True,
                        )

            # Order: kt chunk 0, qt block 0 (the first QK's needs), then
            # kt j=1..3 — putting qt first delays kt[128:512] enough to
            # starve steps 2-3 and re-throttle the PE (measured +3us).
            nc.sync.dma_start(kt[:, 0:128], kt_d[:, 0:128])
            nc.sync.dma_start(qt[:, 0:QB], qt_d[:, 0:QB])
            nc.sync.dma_start(kt[:, 128:512], kt_d[:, 128:512])
            nc.sync.dma_start(vsb[:, :, 0:16, :], v_d[:, :, 0:16, :])
            nc.sync.dma_start(kt[:, 512:], kt_d[:, 512:])
            nc.sync.dma_start(vsb[:, :, 16:, :], v_d[:, :, 16:, :])
            nc.sync.dma_start(qt[:, QB:], qt_d[:, QB:])

            # Software-pipelined emission. PE stream per step is
            #   [PV(s-2) pair1] [QK(s) pair] [PV(s-2) pair2]
            # so that every LDWEIGHTS is two 215ns slots behind the matmul
            # whose weight buffer it reuses (each row group has 3 loads per
            # step and only fg+bg weight slots; an MM's full completion lags
            # its 215ns streaming slot by ~160ns, so a 1-slot distance —
            # QK emitted back-to-back after PV — stalled ~300ns per step).
            steps = [(qb, j) for qb in range(NQB) for j in range(NJ)]
            exs = {}
            pv = {}
            pend = []

            def emit_evac(item):
                h, p0, p1, qb2 = item
                qsl2 = slice(qb2 * QB, (qb2 + 1) * QB)
                ev = evp.tile([65, QB], evdt)
                if EVAC == "split":
                    nc.scalar.copy(ev[:], p0[:, :])
                else:
                    nc.vector.tensor_copy(ev[:], p0[:, :])
                nc.vector.tensor_add(ev[:], ev[:], p1[:, :])
                nc.sync.dma_start(out_d[h, :, qsl2], ev[:])

            def pv_mms(s, pairs):
                qb, j = steps[s]
                exa, exb = exs[s]
                for h, part in pairs:
                    psl = slice(part * 64, (part + 1) * 64)
                    tgt = pv[h, part]
                    first = j == 0
                    last = j == NJ - 1
                    mv = exa[psl, :] if h == 0 else exb[psl, :].bitcast(bf16)
                    nc.tensor.matmul(
                        tgt[:, :],
                        vsb[psl, h, j, :],
                        mv,
                        start=first,
                        stop=last,
                    )

            for s in range(len(steps) + DEPTH):
                if s >= DEPTH:
                    qb2, j2 = steps[s - DEPTH]
                    if j2 == 0:
                        for h in range(2):
                            for part in range(2):
                                pv[h, part] = pvp.tile(
                                    [65, QB], f32,
                                    tag=f"pv_{h}_{part}",
                                    name=f"pv_{h}_{part}",
                                )
                    # pair1: row groups 0 / 64
                    pv_mms(s - DEPTH, ((0, 0), (0, 1)))
                if s < len(steps):
                    qb, j = steps[s]
                    qsl = slice(qb * QB, (qb + 1) * QB)
                    jsl = slice(j * 128, (j + 1) * 128)
                    qka = qkap.tile([128, QB], f32, tag="qka")
                    qkb = qkbp.tile([128, QB], f32, tag="qkb")
                    # head B first: DVE (the slower exp engine) gets its
                    # input one MM-tail (~33ns) earlier.
                    for h, qk in ((1, qkb), (0, qka)):
                        psl = slice(h * 64, (h + 1) * 64)
                        nc.tensor.matmul(
                            qk[:, :],
                            kt[psl, jsl],
                            qt[psl, qsl],
                            start=True,
                            stop=True,
                        )
                    # Head A's exp -> bf16 tile via ACT (true Exp); head B's
                    # -> int16 tile via DVE (Schraudolph). Each engine reads
                    # its own PSUM tile so they run concurrently.
                    exa = exap.tile([128, QB], bf16)
                    exb = exbp.tile([128, QB], i16)
                    nc.vector.tensor_scalar(
                        out=exb[:],
                        in0=qkb[:, :],
                        scalar1=SCH_A,
                        scalar2=SCH_B,
                        op0=MULT,
                        op1=ADD,
                    )
                    nc.scalar.activation(
                        exa[:], qka[:, :], EXP, scale=0.125
                    )
                    exs[s] = (exa, exb)
                # Deferred evacs: one per step so the copy/add pair doesn't
                # burst both heads into the ACT/DVE queues at once (which
                # delayed the exp stream ~1.7us per block boundary). The
                # head-1 evac of block b lands here, which in emission order
                # is BEFORE pair2 of (b+1, j=0) -- i.e. before block b's pv
                # banks are rewritten -- so the read-before-write order the
                # tile tracker derives is correct.
                if pend:
                    emit_evac(pend.pop(0))
                if s >= DEPTH:
                    qb2, j2 = steps[s - DEPTH]
                    # pair2
                    pv_mms(s - DEPTH, ((1, 0), (1, 1)))
                    exs.pop(s - DEPTH)
                    if j2 == NJ - 1:
                        pend.append((0, pv[0, 0], pv[0, 1], qb2))
                        pend.append((1, pv[1, 0], pv[1, 1], qb2))
                        emit_evac(pend.pop(0))
            while pend:
                emit_evac(pend.pop(0))

    nc.compile()
    return nc


def _get_nc():
    if "nc" not in _CACHE:
        _CACHE["nc"] = _build_nc()
    return _CACHE["nc"]


def kernel(target, source, Wq, Wk, Wv, Wo):
    from concourse.bass_utils import run_bass_kernel_spmd

    target = np.asarray(target, dtype=np.float32)
    source = np.asarray(source, dtype=np.float32)
    Wq = np.asarray(Wq, dtype=np.float32)
    Wk = np.asarray(Wk, dtype=np.float32)
    Wv = np.asarray(Wv, dtype=np.float32)
    Wo = np.asarray(Wo, dtype=np.float32)
    B = target.shape[0]

    import ml_dtypes

    bf16 = ml_dtypes.bfloat16
    in_maps = []
    for c in range(NCORES):
        b = c // 2
        h0 = (c % 2) * 2
        cols = slice(h0 * DH, (h0 + 2) * DH)  # 128 cols = 2 heads
        q = target[b] @ Wq[:, cols]           # [S, 128]
        k = source[b] @ Wk[:, cols]           # [S, 128]
        v = source[b] @ Wv[:, cols]           # [S, 128]
        vv = np.ones((128, 2, NJ, 65), bf16)
        # v[p, h, j, 0:64] = V[j*128+p, h*64:(h+1)*64]
        vr = v.reshape(NJ, 128, 2, DH)        # [j, p, h, d]
        vv[:, :, :, :DH] = vr.transpose(1, 2, 0, 3).astype(bf16)
        in_maps.append(
            {
                "qt": np.ascontiguousarray(q.T.astype(bf16)),
                "kt": np.ascontiguousarray(k.T.astype(bf16)),
                "v": vv,
            }
        )

    nc = _get_nc()
    res = run_bass_kernel_spmd(nc, in_maps, core_ids=list(range(NCORES)))

    out = np.zeros((B, S, E), np.float32)
    for c in range(NCORES):
        b = c // 2
        h0 = (c % 2) * 2
        u = np.asarray(res.results[c]["out"], dtype=np.float32)  # [2, 65, S]
        for hh in range(2):
            att_t = u[hh, :DH, :] / u[hh, DH:DH + 1, :]   # [64, S]
            out[b] += att_t.T @ Wo[(h0 + hh) * DH:(h0 + hh + 1) * DH, :]
    return out
